# revision 31
# baseline (speedup 1.0000x reference)
"""nn_ConvModel — Bass/Tile kernel for 8 Trainium2 NeuronCores.

Data-parallel: batch 4096 -> 8 shards of 512. Tiny 3-bit-quantized weights
replicated; each per-tensor fake-quant absmax becomes an AllReduce(max).

All heavy math runs in the integer domain on the tensor engine (fp16
operands holding exact small integers):
  - linear1: lin/k1 = qx . qW1^T + b1/k1, as 84 matmuls with the bias folded
    in as a 29th contraction row (a "ones" column is interleaved into the
    transposed-activation tiles).
  - round-to-nearest-even via fp16 convert at +1536 (fp16 ulp on
    [1024,2048) is exactly 1.0, hardware convert is IEEE RNE).
  - depthwise conv (K=15, pad 7) as 96 block-diagonal banded matmuls over a
    [(4ch x 28pos)+bias, batch] operand layout.
  - final linear accumulated group-wise into a single [10,512] PSUM bank.
"""
import math
import numpy as np

N_CORES = 8
BATCH = 4096
S = BATCH // N_CORES          # 512 per core
C = 384                        # model dim
L = 28                         # sequence length
F = 28                         # input features
K = 15
PAD = 7
OB = C // 128                  # 3 o-blocks
G = C // 4                     # 96 conv groups of 4 channels
NB = S // 128                  # 4 batch chunks
LG = L // 4                    # 7 l-groups of 4

_cache = {}


def _scale_np(absmax, bits=8):
    qmax = np.float32(2 ** (bits - 1) - 1)
    return np.maximum(np.float32(absmax) / qmax, np.float32(1e-8))


def _quant_w(w, bits=3):
    qmax = float(2 ** (bits - 1) - 1)
    qmin = -float(2 ** (bits - 1))
    s = _scale_np(np.abs(w).max(), bits)
    q = np.clip(np.round(np.asarray(w, np.float32) / s), qmin, qmax).astype(np.float32)
    return q, float(s)


def _build_bass(num_devices=N_CORES):
    import concourse.bass as bass
    import concourse.bacc as bacc
    import concourse.mybir as mybir
    import concourse.tile as tile
    from concourse import bass_isa

    dt = mybir.dt
    AF = mybir.ActivationFunctionType
    ALU = mybir.AluOpType
    fp32, fp16 = dt.float32, dt.float16

    nc = bacc.Bacc(num_devices=num_devices)
    groups = [list(range(num_devices))]

    # ---- I/O ----
    img_d = nc.declare_dram_parameter("img", [S, 784], fp32, isOutput=False)
    w1t_d = nc.declare_dram_parameter("w1t", [96, C], fp16, isOutput=False)
    b1r_d = nc.declare_dram_parameter("b1r", [1, C], fp32, isOutput=False)
    tcv_d = nc.declare_dram_parameter("tcv", [113, G * 112], fp16, isOutput=False)
    bcr_d = nc.declare_dram_parameter("bcr", [G, 112], fp32, isOutput=False)
    wft_d = nc.declare_dram_parameter("wft", [112, G * 10], fp16, isOutput=False)
    bfr_d = nc.declare_dram_parameter("bfr", [1, 16], fp32, isOutput=False)
    scl_d = nc.declare_dram_parameter("scl", [128, 8], fp32, isOutput=False)
    eye_d = nc.declare_dram_parameter("eye", [128, 128], fp16, isOutput=False)
    out_d = nc.declare_dram_parameter("out", [S, 10], fp32, isOutput=True)

    M = 1536.0

    with tile.TileContext(nc) as tc:
        with (
            tc.tile_pool(name="wpool", bufs=1) as wpool,
            tc.tile_pool(name="dram", bufs=1, space="DRAM") as dram,
            tc.tile_pool(name="spool", bufs=1) as spool,      # scalar [128,1] tiles
            tc.tile_pool(name="inpool", bufs=1) as inpool,
            tc.tile_pool(name="qpool", bufs=1) as qpool,      # persistent activations
            tc.tile_pool(name="std", bufs=1) as stdpool,      # q1 std-layout per oblock
            tc.tile_pool(name="chain", bufs=2) as chain,      # elementwise chain tiles
            tc.tile_pool(name="psum", bufs=1, space="PSUM") as psum,
            tc.tile_pool(name="psl", bufs=1, space="PSUM") as psl,
        ):
            # ---------- static weights into SBUF ----------
            w1t = wpool.tile([96, C], fp16, tag="w1t")
            eye = wpool.tile([128, 128], fp16, tag="eye")

            bcr = wpool.tile([G, 112], fp32, tag="bcr")
            wft = wpool.tile([112, G * 10], fp16, tag="wft")
            bfr = wpool.tile([1, 16], fp32, tag="bfr")
            scl = wpool.tile([128, 8], fp32, tag="scl")
            nc.sync.dma_start(w1t[:], w1t_d[:])
            nc.sync.dma_start(eye[:], eye_d[:])
            nc.sync.dma_start(bcr[:], bcr_d[:])
            nc.sync.dma_start(wft[:], wft_d[:])
            nc.sync.dma_start(bfr[:], bfr_d[:])
            nc.sync.dma_start(scl[:], scl_d[:])
            sW1 = scl[:, 0:1]
            sWc = scl[:, 1:2]
            sWf = scl[:, 2:3]

            def ts(out, in0, s1v, op0, s2v=None, op1=None):
                if s2v is None:
                    return nc.vector.tensor_scalar(out, in0, s1v, None, op0)
                return nc.vector.tensor_scalar(out, in0, s1v, s2v, op0, op1)

            # small helper: AllReduce(max) of a [128,1] fp32 SBUF tile
            def armax(sb_tile, name):
                din = dram.tile([128, 1], fp32, tag=f"arin{name}")
                dout = dram.tile([128, 1], fp32, tag=f"arout{name}")
                nc.sync.dma_start(din[:], sb_tile[:])
                nc.gpsimd.collective_compute(
                    "AllReduce", ALU.max, replica_groups=groups,
                    ins=[din.opt()], outs=[dout.opt()],
                )
                res = spool.tile([128, 1], fp32, tag=f"arres{name}")
                nc.sync.dma_start(res[:], dout[:])
                return res

            # ---------- phase 0: load image, s0, quantize, transpose ----------
            imgr = img_d.rearrange("(n p) f -> n p f", p=128)
            mx0 = spool.tile([128, NB], fp32, tag="mx0")
            for n in range(NB):
                img = inpool.tile([128, 784], fp32, tag="img", bufs=2)
                nc.sync.dma_start(img[:], imgr[n:n+1])
                nc.vector.tensor_reduce(mx0[:, n:n + 1], img[:],
                                        mybir.AxisListType.X,
                                        ALU.max, apply_absolute_value=True)
            m0 = spool.tile([128, 1], fp32, tag="m0")
            nc.vector.tensor_reduce(m0[:], mx0[:], mybir.AxisListType.X, ALU.max)
            m0r = spool.tile([128, 1], fp32, tag="m0r")
            nc.gpsimd.partition_all_reduce(m0r[:], m0[:], channels=128,
                                           reduce_op=bass_isa.ReduceOp.max)
            m0g = armax(m0r, "0")
            s0 = spool.tile([128, 1], fp32, tag="s0")
            ts(s0[:], m0g[:], 1.0 / 127.0, ALU.mult, 1e-8, ALU.max)
            c0 = spool.tile([128, 1], fp32, tag="c0")
            nc.vector.reciprocal(c0[:], s0[:])
            cM = spool.tile([128, 1], fp32, tag="cM")
            nc.vector.memset(cM[:], M)

            # qx interleaved with ones/pad columns: [128, NB, 28l, 32]
            # shares the "big" slot with q1s (qxi dead before q1s is born)
            qxi = stdpool.tile([128, NB, L, 32], fp16, tag="big", name="qxi")
            for n in range(NB):
                img = inpool.tile([128, 784], fp32, tag="img", bufs=2)
                nc.sync.dma_start(img[:], imgr[n:n+1])
                ts(qxi[:, n, :, 0:28], img.rearrange("p (l f) -> p l f", f=28),
                   c0[:], ALU.mult, M, ALU.add)
            nc.vector.memset(qxi[:, :, :, 28:29], M + 1.0)
            nc.vector.memset(qxi[:, :, :, 29:32], M)

            # w1t bias row: b1/k1 = b1 * c0 / sW1  (fp16) -> rows 28/60/92
            b1r = wpool.tile([1, C], fp32, tag="b1r")
            nc.sync.dma_start(b1r[:], b1r_d[:])
            b1k = wpool.tile([1, C], fp16, tag="b1k")
            t0 = spool.tile([128, 1], fp32, tag="t0")
            nc.vector.reciprocal(t0[:], sW1[:])
            t1 = spool.tile([128, 1], fp32, tag="t1")
            ts(t1[:], t0[:], c0[:], ALU.mult)
            ts(b1k[:], b1r[:], t1[0:1, :], ALU.mult)
            for m in range(3):
                nc.sync.dma_start(w1t[32 * m + 28:32 * m + 29, :], b1k[:])

            # transpose qx -> qxT tiles [96, S] fp16 (rows: 3l x (28f+one+3pad))
            NT = 10  # 9 tiles of 3 l's + 1 tile of 1 l
            qxT = [inpool.tile([96, S], fp16, tag=f"qxT{t}", name=f"qxT{t}")
                   for t in range(NT)]
            for t in range(NT):
                nl = min(3, L - 3 * t)
                for n in range(NB):
                    pt = psum.tile([96, 128], fp16, tag="ptr", bufs=2)
                    nc.tensor.transpose(
                        pt[0:32 * nl, :], qxi[:, n, 3 * t:3 * t + nl, :], eye[:])
                    ts(qxT[t][0:32 * nl, n * 128:(n + 1) * 128],
                       pt[0:32 * nl, :], M, ALU.subtract)

            # ---------- phase 1: mm1 pass 1 -> max|lin|/k1 ----------
            mx1 = spool.tile([128, OB * 14], fp32, tag="mx1")
            for ob in range(OB):
                for q in range(14):
                    ps = psum.tile([128, 2, 512], fp32, tag="mm", bufs=2)
                    for i in range(2):
                        l, m = 2 * q + i, (2 * q + i) % 3
                        nc.tensor.matmul(
                            ps[:, i, :],
                            w1t[32 * m:32 * m + 29, ob * 128:(ob + 1) * 128],
                            qxT[l // 3][32 * m:32 * m + 29, :],
                            start=True, stop=True)
                    nc.vector.tensor_reduce(
                        mx1[:, ob * 14 + q: ob * 14 + q + 1], ps[:],
                        mybir.AxisListType.XY, ALU.max, apply_absolute_value=True)
            m1 = spool.tile([128, 1], fp32, tag="m1")
            nc.vector.tensor_reduce(m1[:], mx1[:], mybir.AxisListType.X, ALU.max)
            m1r = spool.tile([128, 1], fp32, tag="m1r")
            nc.gpsimd.partition_all_reduce(m1r[:], m1[:], channels=128,
                                           reduce_op=bass_isa.ReduceOp.max)
            m1g = armax(m1r, "1")

            # scalar chain for stage 1
            s1 = spool.tile([128, 1], fp32, tag="s1")
            ts(s1[:], m1g[:], s0[:], ALU.mult)
            ts(s1[:], s1[:], sW1[:], ALU.mult)
            ts(s1[:], s1[:], 1.0 / 127.0, ALU.mult, 1e-8, ALU.max)
            r1 = spool.tile([128, 1], fp32, tag="r1")
            nc.vector.reciprocal(r1[:], s1[:])
            a1 = spool.tile([128, 1], fp32, tag="a1")
            ts(a1[:], s0[:], sW1[:], ALU.mult)
            ts(a1[:], a1[:], r1[:], ALU.mult)
            th1 = spool.tile([128, 1], fp32, tag="th1")
            nc.scalar.activation(th1[:], s1[:], AF.Tanh, scale=127.0)
            s2 = spool.tile([128, 1], fp32, tag="s2")
            ts(s2[:], th1[:], 1.0 / 127.0, ALU.mult, 1e-8, ALU.max)
            b1s = spool.tile([128, 1], fp32, tag="b1s")
            nc.vector.reciprocal(b1s[:], s2[:])
            nb1 = spool.tile([128, 1], fp32, tag="nb1")
            ts(nb1[:], s1[:], -M, ALU.mult)

            # conv weight bias row: bc/k3 (after s2 known)
            ck3 = spool.tile([128, 1], fp32, tag="ck3")
            ts(ck3[:], s2[:], sWc[:], ALU.mult)
            nc.vector.reciprocal(ck3[:], ck3[:])
            bck = wpool.tile([G, 112], fp16, tag="bck")
            ts(bck[:], bcr[:], ck3[0:G, :], ALU.mult)

            def load_tcv(ob):
                t = wpool.tile([113, 32 * 112], fp16, tag="tcvt", bufs=2,
                               name=f"tcvt")
                nc.sync.dma_start(
                    t[0:112, :], tcv_d[0:112, ob * 3584:(ob + 1) * 3584])
                nc.sync.dma_start(
                    t[112:113, :], bck[32 * ob:32 * ob + 32, :])
                return t

            # ---------- phase 2: mm1 pass 2 + quant chain + conv + max ----------
            q1c = qpool.tile([113, G * 512], fp16, tag="q1c")
            nc.gpsimd.memset(q1c[96:113, :], 1.0)
            mx3 = spool.tile([128, G // 2], fp32, tag="mx3")
            for ob in range(OB):
                q1s = stdpool.tile([128, L, 512], fp16, tag="big", name="q1s")
                for q in range(14):
                    ps = psum.tile([128, 2, 512], fp32, tag="mm", bufs=2)
                    for i in range(2):
                        l, m = 2 * q + i, (2 * q + i) % 3
                        nc.tensor.matmul(
                            ps[:, i, :],
                            w1t[32 * m:32 * m + 29, ob * 128:(ob + 1) * 128],
                            qxT[l // 3][32 * m:32 * m + 29, :],
                            start=True, stop=True)
                    tq = chain.tile([128, 2, 512], fp16, tag="tq")
                    nc.scalar.activation(tq[:], ps[:], AF.Identity,
                                         bias=cM[:], scale=a1[:])
                    tw = chain.tile([128, 2, 512], fp32, tag="tw")
                    nc.scalar.activation(tw[:], tq[:], AF.Tanh,
                                         bias=nb1[:], scale=s1[:])
                    tv = chain.tile([128, 2, 512], fp16, tag="tv")
                    ts(tv[:], tw[:], b1s[:], ALU.mult, M, ALU.add)
                    ts(q1s[:, 2 * q:2 * q + 2, :], tv[:], M, ALU.subtract)
                # layout conversion: q1s [c128,(l,b)] -> q1c [(c4,l)112,(g,b)]
                for j in range(32):
                    g = ob * 32 + j
                    for i in range(4):
                        nc.sync.dma_start(
                            q1c[28 * i:28 * i + 28, g * 512:(g + 1) * 512],
                            q1s[4 * j + i:4 * j + i + 1, :, :])
                # conv pass 1 (max only) for this oblock
                tcvt = load_tcv(ob)
                for w in range(16):
                    pc = psum.tile([112, 2, 512], fp32, tag="mm", bufs=2)
                    for i in range(2):
                        g = ob * 32 + 2 * w + i
                        gl = 2 * w + i
                        nc.tensor.matmul(
                            pc[:, i, :], tcvt[:, gl * 112:(gl + 1) * 112],
                            q1c[:, g * 512:(g + 1) * 512], start=True, stop=True)
                    nc.vector.tensor_reduce(
                        mx3[0:112, ob * 16 + w: ob * 16 + w + 1], pc[:],
                        mybir.AxisListType.XY, ALU.max, apply_absolute_value=True)
            m3 = spool.tile([128, 1], fp32, tag="m3")
            nc.vector.tensor_reduce(m3[0:112, :], mx3[0:112, :],
                                    mybir.AxisListType.X, ALU.max)
            m3r = spool.tile([128, 1], fp32, tag="m3r")
            nc.gpsimd.partition_all_reduce(m3r[0:112, :], m3[0:112, :], channels=112,
                                           reduce_op=bass_isa.ReduceOp.max)
            m3b = spool.tile([128, 1], fp32, tag="m3b")
            nc.gpsimd.partition_broadcast(m3b[:], m3r[0:1, :])
            m3g = armax(m3b, "3")

            s3 = spool.tile([128, 1], fp32, tag="s3")
            ts(s3[:], m3g[:], s2[:], ALU.mult)
            ts(s3[:], s3[:], sWc[:], ALU.mult)
            ts(s3[:], s3[:], 1.0 / 127.0, ALU.mult, 1e-8, ALU.max)
            r3 = spool.tile([128, 1], fp32, tag="r3")
            nc.vector.reciprocal(r3[:], s3[:])
            a3 = spool.tile([128, 1], fp32, tag="a3")
            ts(a3[:], s2[:], sWc[:], ALU.mult)
            ts(a3[:], a3[:], r3[:], ALU.mult)
            th3 = spool.tile([128, 1], fp32, tag="th3")
            nc.scalar.activation(th3[:], s3[:], AF.Tanh, scale=127.0)
            s4 = spool.tile([128, 1], fp32, tag="s4")
            ts(s4[:], th3[:], 1.0 / 127.0, ALU.mult, 1e-8, ALU.max)
            b3s = spool.tile([128, 1], fp32, tag="b3s")
            nc.vector.reciprocal(b3s[:], s4[:])
            nb3 = spool.tile([128, 1], fp32, tag="nb3")
            ts(nb3[:], s3[:], -M, ALU.mult)

            # bf/k5 row (fp16) for the final-bias matmul
            ck5 = spool.tile([128, 1], fp32, tag="ck5")
            ts(ck5[:], s4[:], sWf[:], ALU.mult)
            nc.vector.reciprocal(ck5[:], ck5[:])
            bfk = wpool.tile([1, 16], fp16, tag="bfk")
            ts(bfk[:], bfr[:], ck5[0:1, :], ALU.mult)

            # ---------- phase 3: conv pass 2 + chain + final linear ----------
            ones = wpool.tile([1, 512], fp16, tag="ones")
            nc.vector.memset(ones[:], 1.0)
            pl = psl.tile([10, 512], fp32, tag="pl")
            nc.tensor.matmul(pl[:], bfk[:, 0:10], ones[:],
                             start=True, stop=False)
            for w in range(G // 2):
                if w % 16 == 0:
                    tcvt = load_tcv(w // 16)
                pc = psum.tile([112, 2, 512], fp32, tag="mm", bufs=2)
                for i in range(2):
                    g = 2 * w + i
                    gl = g - (w // 16) * 32
                    nc.tensor.matmul(
                        pc[:, i, :], tcvt[:, gl * 112:(gl + 1) * 112],
                        q1c[:, g * 512:(g + 1) * 512], start=True, stop=True)
                tq = chain.tile([112, 2, 512], fp16, tag="tq")
                nc.scalar.activation(tq[:], pc[:], AF.Identity,
                                     bias=cM[0:112, :], scale=a3[0:112, :])
                tw = chain.tile([112, 2, 512], fp32, tag="tw")
                nc.scalar.activation(tw[:], tq[:], AF.Tanh,
                                     bias=nb3[0:112, :], scale=s3[0:112, :])
                tv = chain.tile([112, 2, 512], fp16, tag="tv")
                ts(tv[:], tw[:], b3s[0:112, :], ALU.mult, M, ALU.add)
                q2 = chain.tile([112, 2, 512], fp16, tag="q23")
                ts(q2[:], tv[:], M, ALU.subtract)
                for i in range(2):
                    g = 2 * w + i
                    nc.tensor.matmul(
                        pl[:], wft[:, g * 10:(g + 1) * 10], q2[:, i, :],
                        start=False, stop=(g == G - 1))

            mx5 = spool.tile([128, 1], fp32, tag="mx5")
            nc.vector.tensor_reduce(mx5[0:10, :], pl[:], mybir.AxisListType.X,
                                    ALU.max, apply_absolute_value=True)
            m5r = spool.tile([128, 1], fp32, tag="m5r")
            nc.gpsimd.partition_all_reduce(m5r[0:10, :], mx5[0:10, :], channels=10,
                                           reduce_op=bass_isa.ReduceOp.max)
            m5b = spool.tile([128, 1], fp32, tag="m5b")
            nc.gpsimd.partition_broadcast(m5b[:], m5r[0:1, :])
            m5g = armax(m5b, "5")

            s5 = spool.tile([128, 1], fp32, tag="s5")
            ts(s5[:], m5g[:], s4[:], ALU.mult)
            ts(s5[:], s5[:], sWf[:], ALU.mult)
            ts(s5[:], s5[:], 1.0 / 127.0, ALU.mult, 1e-8, ALU.max)
            r5 = spool.tile([128, 1], fp32, tag="r5")
            nc.vector.reciprocal(r5[:], s5[:])
            a5 = spool.tile([128, 1], fp32, tag="a5")
            ts(a5[:], s4[:], sWf[:], ALU.mult)
            ts(a5[:], a5[:], r5[:], ALU.mult)

            lq = chain.tile([10, 512], fp16, tag="tq")
            nc.scalar.activation(lq[:], pl[:], AF.Identity, bias=cM[0:10, :],
                                 scale=a5[0:10, :])
            li = chain.tile([10, 512], fp16, tag="q23")
            ts(li[:], lq[:], M, ALU.subtract)
            outt = chain.tile([128, NB, 10], fp32, tag="outt")
            for n in range(NB):
                pt = psum.tile([128, 10], fp16, tag="ptr", bufs=2, name="ptl")
                nc.tensor.transpose(pt[:], li[:, n * 128:(n + 1) * 128],
                                    eye[0:10, 0:10])
                ts(outt[:, n, :], pt[:], s5[:], ALU.mult)
            nc.sync.dma_start(out_d.rearrange("(n p) o -> p n o", p=128), outt[:])

    nc.compile()
    return nc


def _prep_host(W1, b1, Wc, bc, Wf, bf):
    """Host-side: quantize weights, build the derived operand tensors."""
    qW1, sW1 = _quant_w(W1)            # [384, 28]
    qWc, sWc = _quant_w(Wc)            # [384, 1, 15]
    qWf, sWf = _quant_w(Wf)            # [10, 10752]

    w1t = np.zeros((96, C), np.float16)
    for m in range(3):
        w1t[32 * m:32 * m + 28, :] = qW1.T.astype(np.float16)

    b1r = np.asarray(b1, np.float32).reshape(1, C)

    # conv block-diag lhsT: [113, G*112]; col (g, c*28 + l_out), row (c*28 + l_in)
    tcv = np.zeros((113, G, 112), np.float16)
    wc = qWc[:, 0, :]  # [384, 15]
    band = np.zeros((28, 28), np.float32)
    for g in range(G):
        for c4 in range(4):
            ch = 4 * g + c4
            band[:] = 0.0
            for li in range(28):
                lo0 = max(0, li - PAD)
                lo1 = min(28, li + PAD + 1)
                for lo in range(lo0, lo1):
                    band[li, lo] = wc[ch, li - lo + PAD]
            tcv[28 * c4:28 * c4 + 28, g, 28 * c4:28 * c4 + 28] = band.astype(np.float16)
    tcv = tcv.reshape(113, G * 112)

    bcr = np.zeros((G, 112), np.float32)
    for g in range(G):
        for p in range(112):
            bcr[g, p] = bc[4 * g + p // 28]

    # wft[p=(c*28+l), (g,10)] = qWf[:, l*384 + 4g+c]
    wft = np.zeros((112, G, 10), np.float16)
    qWf3 = qWf.reshape(10, 28, 384)  # [10, l, ch]
    for g in range(G):
        for c4 in range(4):
            for l in range(28):
                wft[28 * c4 + l, g, :] = qWf3[:, l, 4 * g + c4].astype(np.float16)
    wft = wft.reshape(112, G * 10)

    bfr = np.zeros((1, 16), np.float32)
    bfr[0, :10] = np.asarray(bf, np.float32)

    scl = np.zeros((128, 8), np.float32)
    scl[:, 0] = sW1
    scl[:, 1] = sWc
    scl[:, 2] = sWf

    eye = np.eye(128, dtype=np.float16)
    return dict(w1t=w1t, b1r=b1r, tcv=tcv, bcr=bcr, wft=wft, bfr=bfr,
                scl=scl, eye=eye)


def _run(image, W1, b1, Wc, bc, Wf, bf, **kwargs):
    from concourse.bass_utils import run_bass_kernel_spmd

    if "nc" not in _cache:
        _cache["nc"] = _build_bass()
    nc = _cache["nc"]

    host = _prep_host(W1, b1, Wc, bc, Wf, bf)
    img = np.ascontiguousarray(
        np.asarray(image, np.float32).reshape(BATCH, 784))
    in_maps = []
    for c in range(N_CORES):
        m = dict(host)
        m["img"] = img[c * S:(c + 1) * S]
        in_maps.append(m)

    res = run_bass_kernel_spmd(nc, in_maps, core_ids=list(range(N_CORES)),
                               **kwargs)
    out = np.concatenate([res.results[c]["out"] for c in range(N_CORES)], axis=0)
    return np.ascontiguousarray(out.astype(np.float32)), res


def kernel(image, W1, b1, Wc, bc, Wf, bf):
    out, _ = _run(image, W1, b1, Wc, bc, Wf, bf)
    return out
